# revision 77
# baseline (speedup 1.0000x reference)
"""Trainium2 Bass kernel for nn_Damping (B=32768, N=64, H=256).

Per-sample computation:
    diag = (relu(MLP_d(x)) + damp_min) * x          # [64]
    off  = MLP_o(x)                                  # [2016] strictly-lower entries
    L    = scatter(off -> strict lower, diag -> diagonal)   # [64, 64]
    out  = L @ (L^T @ x)

Strategy: pure data parallel over 8 NeuronCores (4096 samples each).
On-chip layout is feature-major: x arrives pre-transposed from the host as
bf16 [64, 4096] and the output leaves feature-major [64, 4096] f32 (host
transposes back), so the device does zero PE transposes. The scatter
matvecs avoid materializing L:
    v   = Ecol^T @ (off * (Rrow @ xT)) + diag * x       (v = L^T x)
    out = Erow^T @ (off * (Rcol @ vT)) + diag * v       (out = L v)
with Rrow/Rcol 0/1 expansion matrices and Ecol/Erow 0/1 reduction matrices
(PE matmuls, fp32 PSUM accumulation). All matmul operands are bf16.

Per 512-sample block: 110 matmul passes (free=512). Emission is software-
pipelined so the PE queue never head-of-line blocks on the DVE multiplies:
reduction matmuls for slice-pair q are emitted after the independent
woo/expand matmuls of pair q+1. Elementwise work is split DVE (scatter
multiplies, PSUM-reading adds) / Act (PSUM->SBUF off copies + tanh) /
GpSimd (SBUF-only diag-path ops).
"""

import numpy as np

B, N, H, OFF = 32768, 64, 256, 2016
NCORES = 8
BLOCAL = B // NCORES          # 4096 samples per core
NSLICES = 16
SL = 128                      # padded slice width; 16*128 = 2048
OFFP = NSLICES * SL           # 2048 (padded off dim)
NBLOCKS = 8                   # blocks of 512 samples per core
BT = 512                      # batch tile (moving free dim)
NPAIRS = NSLICES // 2         # slice pairs for the paired DVE multiplies

_compiled = {}


def _build_program(with_boo=True):
    import concourse.bass as bass  # noqa: F401
    import concourse.mybir as mybir
    import concourse.tile as tile
    from concourse import bacc

    f32 = mybir.dt.float32
    bf16 = mybir.dt.bfloat16
    AF = mybir.ActivationFunctionType

    nc = bacc.Bacc("TRN2", target_bir_lowering=False, debug=False,
                   num_devices=NCORES)

    def din(name, shape, dt=f32):
        return nc.dram_tensor(name, list(shape), dt, kind="ExternalInput").ap()

    xt_ap = din("xt", (128, BLOCAL), bf16)     # bottom 64 partitions zero
    # first-use pack: wd1 | wo1 | xt block 0 (one DMA before the first matmul)
    w1x_ap = din("w1x", (128, 2 * H + BT), bf16)
    xe1_ap = din("xe1", (SL, NSLICES, BLOCAL), bf16)
    wd2_ap = din("wd2", (128, 2, H), bf16)
    wdo_ap = din("wdo", (128, 2, 128), bf16)   # out cols 64-127 zero
    wo2_ap = din("wo2", (128, 2, H), bf16)
    woo_ap = din("woo", (128, 2, OFFP), bf16)
    # small consts packed: cols 0-1 bd1, 2-3 bo1, 4-5 bd2, 6-7 bo2, 8 bdo,
    # 9..521 dmf (bdo/dmf live on partitions 0-63)
    blob_ap = din("blob", (128, 9 + BT))
    # b1 = Ecol^T diag(boo) Rrow, b2 = Erow^T diag(boo) Rcol (both padded)
    blobb_ap = din("blobb", (128, 256), bf16)
    rcol_ap = din("rcol", (128, OFFP), bf16)   # bottom 64 rows zero
    ecol_ap = din("ecol", (SL, NSLICES * 128), bf16)  # out cols 64-127 zero
    erow_ap = din("erow", (SL, NSLICES * 128), bf16)
    out_ap = nc.dram_tensor("out", [N, BLOCAL], f32, kind="ExternalOutput").ap()

    with tile.TileContext(nc) as tc:
        with (
            tc.tile_pool(name="consts", bufs=1) as consts,
            tc.tile_pool(name="acts", bufs=2) as act_pool,
            tc.tile_pool(name="offp", bufs=2) as off_pool,
            tc.tile_pool(name="mp", bufs=6) as m_pool,
            tc.tile_pool(name="small", bufs=2) as small_pool,
            tc.tile_pool(name="outp", bufs=2) as out_pool,
            tc.tile_pool(name="xe1", bufs=3) as xe_pool,
            # PSUM: 8 banks of [128, 512] f32 total.
            tc.tile_pool(name="ps_a", bufs=2, space="PSUM") as ps_a,      # 2
            tc.tile_pool(name="ps_big", bufs=2, space="PSUM") as ps_big,  # 4
            tc.tile_pool(name="ps_acc", bufs=2, space="PSUM") as ps_acc,  # 2
        ):
            # ---- load constants ----
            _ld_engines = [nc.sync, nc.scalar]
            _ld_n = [0]

            def load(name, shape, ap):
                t = consts.tile(list(shape), ap.dtype, tag=name, name=name)
                _ld_engines[_ld_n[0] % 2].dma_start(t[:], ap)
                _ld_n[0] += 1
                return t

            # Loads ordered by first use so the PE can start ~immediately.
            xts = []

            def load_xt(b):
                t = consts.tile([128, BT], bf16, tag=f"xt{b}", name=f"xt{b}")
                nc.sync.dma_start(t[:], xt_ap[:, BT * b:BT * (b + 1)])
                xts.append(t)

            # double-buffered HBM-precomputed pass-1 expansion tiles
            xe_tiles = [None] * NBLOCKS

            def prefetch_xe(b):
                if b < NBLOCKS:
                    t = xe_pool.tile([SL, NSLICES, BT], bf16, tag="xe1")
                    nc.sync.dma_start(t[:], xe1_ap[:, :, BT * b:BT * (b + 1)])
                    xe_tiles[b] = t

            w1x = load("w1x", (128, 2 * H + BT), w1x_ap)
            wd1 = w1x[:, 0:H]
            wo1 = w1x[:, H:2 * H]
            xts.append(w1x[:, 2 * H:2 * H + BT])
            blob = load("blob", (128, 9 + BT), blob_ap)
            wd2 = load("wd2", (128, 2, H), wd2_ap)
            wo2 = load("wo2", (128, 2, H), wo2_ap)
            wdo = load("wdo", (128, 2, 128), wdo_ap)
            woo = load("woo", (128, 2, OFFP), woo_ap)
            blobb = load("blobb", (128, 256), blobb_ap)
            ecol = load("ecol", (SL, NSLICES * 128), ecol_ap)
            prefetch_xe(0)
            load_xt(1)
            rcol = load("rcol", (128, OFFP), rcol_ap)
            erow = load("erow", (SL, NSLICES * 128), erow_ap)
            prefetch_xe(1)
            for _b in range(2, NBLOCKS):
                load_xt(_b)
            bd1, bo1 = blob[:, 0:2], blob[:, 2:4]
            bd2, bo2 = blob[:, 4:6], blob[:, 6:8]
            bdo = blob[0:N, 8:9]
            dmf = blob[0:N, 9:9 + BT]
            b1, b2 = blobb[:, 0:128], blobb[:, 128:256]

            # v tiles: [128, BT] with the bottom 64 partitions kept zero so
            # the zero-padded 128-row rcol stationaries see finite data.
            vts = [consts.tile([128, BT], bf16, tag=f"v{i}", name=f"v{i}")
                   for i in (0, 1)]
            for vt in vts:
                nc.vector.tensor_copy(vt[N:128, :], xts[0][N:128, :])

            def mlp2(w1, b1, w2, b2, xT, tag):
                """Two tanh layers; returns [128, 2, 512] feature-major bf16.

                Emits only the L1 matmuls + activations; L2 is a second call
                so the two MLPs' matmuls interleave (PE never waits on tanh).
                """
                a1 = act_pool.tile([128, 2, BT], bf16, tag=tag + "1")
                for s in range(2):
                    ps = ps_a.tile([128, BT], f32, tag="mlp")
                    nc.tensor.matmul(ps[:], w1[:, 128 * s:128 * (s + 1)],
                                     xT, start=True, stop=True)
                    nc.scalar.activation(a1[:, s], ps[:], AF.Tanh,
                                         bias=b1[:, s:s + 1])
                a2 = act_pool.tile([128, 2, BT], bf16, tag=tag + "2")
                for s in range(2):
                    ps = ps_a.tile([128, BT], f32, tag="mlp")
                    for k in range(2):
                        nc.tensor.matmul(ps[:], w2[:, k, 128 * s:128 * (s + 1)],
                                         a1[:, k], start=(k == 0), stop=(k == 1))
                    nc.scalar.activation(a2[:, s], ps[:], AF.Tanh,
                                         bias=b2[:, s:s + 1])
                return a2

            def scatter_pass1(off, xe, g2, acc_ps, mov, pending=None):
                """off = Woo@g2 (boo folded into acc via B1);
                acc = Ecol^T (off * xe) + B1 @ x. The pass-1 expansion xe is
                precomputed on the host and streamed from HBM, so the multiply
                is all-SBUF bf16 (2x DVE mode). Reduction matmuls for pair q
                are emitted inside iteration q+1 so the PE queue doesn't block
                on the DVE."""
                if with_boo:
                    nc.tensor.matmul(acc_ps[:], b1, mov,
                                     start=True, stop=False)
                m1s = [None] * NPAIRS
                for q in range(NPAIRS):
                    if q == 1 and pending is not None:
                        pending()
                    pso = ps_big.tile([128, 2 * BT], f32, tag="big")
                    for j in range(2):
                        s = 2 * q + j
                        for k in range(2):
                            nc.tensor.matmul(
                                pso[:, BT * j:BT * (j + 1)],
                                woo[:, k, SL * s:SL * (s + 1)],
                                g2[:, k], start=(k == 0), stop=(k == 1))
                    nc.scalar.copy(off[:, 2 * q:2 * q + 2], pso[:])
                    # reductions delayed by two pairs (PE never waits on DVE)
                    if q > 1:
                        for j in range(2):
                            s = 2 * (q - 2) + j
                            nc.tensor.matmul(
                                acc_ps[:], ecol[:, 128 * s:128 * (s + 1)],
                                m1s[q - 2][:, BT * j:BT * (j + 1)],
                                start=(not with_boo and s == 0), stop=False)
                    m1 = m_pool.tile([128, 2 * BT], bf16, tag="m1")
                    m1s[q] = m1
                    nc.vector.tensor_mul(out=m1[:], in0=off[:, 2 * q:2 * q + 2],
                                         in1=xe[:, 2 * q:2 * q + 2])

                def finish():
                    for q in (NPAIRS - 2, NPAIRS - 1):
                        for j in range(2):
                            s = 2 * q + j
                            nc.tensor.matmul(
                                acc_ps[:], ecol[:, 128 * s:128 * (s + 1)],
                                m1s[q][:, BT * j:BT * (j + 1)],
                                start=False, stop=(s == OFFP // SL - 1))
                return finish

            def scatter_pass2(off, mov, acc_ps):
                """acc = Erow^T (off * (Rcol @ mov)) + B2 @ mov."""
                if with_boo:
                    nc.tensor.matmul(acc_ps[:], b2, mov,
                                     start=True, stop=False)
                m1s = [None] * NPAIRS
                for q in range(NPAIRS):
                    pse = ps_big.tile([128, 2 * BT], f32, tag="big")
                    for j in range(2):
                        s = 2 * q + j
                        nc.tensor.matmul(
                            pse[:, BT * j:BT * (j + 1)],
                            rcol[:, SL * s:SL * (s + 1)],
                            mov, start=True, stop=True)
                    if q > 1:
                        for j in range(2):
                            s = 2 * (q - 2) + j
                            nc.tensor.matmul(
                                acc_ps[:], erow[:, 128 * s:128 * (s + 1)],
                                m1s[q - 2][:, BT * j:BT * (j + 1)],
                                start=(not with_boo and s == 0), stop=False)
                    m1 = m_pool.tile([128, 2 * BT], bf16, tag="m2")
                    m1s[q] = m1
                    nc.vector.tensor_mul(out=m1[:], in0=off[:, 2 * q:2 * q + 2],
                                         in1=pse[:])

                def finish():
                    for q in (NPAIRS - 2, NPAIRS - 1):
                        for j in range(2):
                            s = 2 * q + j
                            nc.tensor.matmul(
                                acc_ps[:], erow[:, 128 * s:128 * (s + 1)],
                                m1s[q][:, BT * j:BT * (j + 1)],
                                start=False, stop=(s == OFFP // SL - 1))
                return finish

            def mlp_block(b, pending=None):
                """Both MLPs for block b (matmuls interleaved). `pending`
                (deferred tail reductions of the previous pass) is emitted
                between the two MLPs so those matmuls never head-of-line
                block the PE queue while their DVE inputs finish."""
                xT = xts[b][:]
                h2 = mlp2(wd1, bd1, wd2, bd2, xT, "h")
                if pending is not None:
                    pending()
                g2 = mlp2(wo1, bo1, wo2, bo2, xT, "g")
                return h2, g2

            def diag_block(b, h2):
                """diag = (relu(d + bdo) + dm) * x  (fp32). Depends only on
                h2(b); emitted right after mlp_block(b) so the wdo matmuls
                add independent PE fill at the pass-1/pass-2 boundary."""
                xTn = xts[b][0:N, :]
                psd = ps_a.tile([128, BT], f32, tag="mlp")
                for k in range(2):
                    nc.tensor.matmul(psd[:], wdo[:, k, :], h2[:, k],
                                     start=(k == 0), stop=(k == 1))
                dr = small_pool.tile([N, BT], f32, tag="dr")
                nc.scalar.activation(dr[:], psd[0:N, :], AF.Relu, bias=bdo)
                dd = small_pool.tile([N, BT], f32, tag="dd")
                nc.gpsimd.tensor_add(out=dd[:], in0=dr[:], in1=dmf)
                diag = small_pool.tile([N, BT], f32, tag="diag")
                nc.gpsimd.tensor_mul(out=diag[:], in0=dd[:], in1=xTn)
                dvx = small_pool.tile([N, BT], f32, tag="dvx")
                nc.gpsimd.tensor_mul(out=dvx[:], in0=diag[:], in1=xTn)
                return diag, dvx

            mlps = mlp_block(0)
            diags = diag_block(0, mlps[0])
            fin2 = None
            for b in range(NBLOCKS):
                xT = xts[b][:]                          # [128, BT], bottom 0
                h2, g2 = mlps
                diag, dvx = diags
                prefetch_xe(b + 2)

                # ---- pass 1: v = Ecol^T (off * xe) + B1 x + diag*x ----
                off = off_pool.tile([SL, NSLICES, BT], bf16, tag="off")
                psv = ps_acc.tile([128, BT], f32, tag="acc")
                fin1 = scatter_pass1(off, xe_tiles[b], g2, psv, xT,
                                     pending=fin2)

                # next block's MLP matmuls fill the PE while v is assembled;
                # pass-1 tail reductions are emitted inside (never at queue
                # head while their DVE multiplies finish)
                if b + 1 < NBLOCKS:
                    mlps = mlp_block(b + 1, pending=fin1)
                    diags = diag_block(b + 1, mlps[0])
                else:
                    fin1()
                v = vts[b % 2]
                nc.vector.tensor_add(out=v[0:N, :], in0=psv[0:N, :],
                                     in1=dvx[:])

                # ---- pass 2: out = Erow^T (off * (Rcol vT)) + B2 v + diag*v
                pso2 = ps_acc.tile([128, BT], f32, tag="acc")
                fin2t = scatter_pass2(off, v[:], pso2)
                dvv = small_pool.tile([N, BT], f32, tag="dvv")
                nc.gpsimd.tensor_mul(out=dvv[:], in0=diag[:], in1=v[0:N, :])

                def out_emit(b=b, pso2=pso2, dvv=dvv, fin2t=fin2t):
                    fin2t()   # close the pso2 accumulation group first
                    outf = out_pool.tile([N, BT], f32, tag="outf",
                                         name="outf")
                    nc.vector.tensor_add(out=outf[:], in0=pso2[0:N, :],
                                         in1=dvv[:])
                    nc.sync.dma_start(out_ap[:, BT * b:BT * (b + 1)],
                                      outf[:])

                if b == NBLOCKS - 1:
                    out_emit()
                else:
                    fin2 = out_emit

    nc.compile()
    return nc


def _get_program(with_boo=True):
    if with_boo not in _compiled:
        _compiled[with_boo] = _build_program(with_boo)
    return _compiled[with_boo]


def _host_consts(inputs):
    import ml_dtypes
    f = np.float32
    bf = ml_dtypes.bfloat16
    rows, cols = np.tril_indices(N, k=-1)         # length 2016
    # padded index arrays: entries p >= 2016 are dead (all matrices zero there)
    npad = OFFP - len(rows)                        # 32

    def onehot(idx, num, valid):
        m = np.zeros((num, OFFP), f)
        m[idx[valid], np.where(valid)[0]] = 1.0
        return m

    valid = np.ones(OFFP, bool)
    valid[len(rows):] = False
    cols_p = np.concatenate([cols, np.zeros(npad, int)])

    rcol = np.zeros((128, OFFP), f)
    rcol[:N] = onehot(cols_p, N, valid)           # padded [128, 2048]
    ecol = np.zeros((SL, NSLICES, 128), f)
    erow = np.zeros((SL, NSLICES, 128), f)
    for s in range(NSLICES):
        for m in range(SL):
            p = SL * s + m
            if p < len(rows):
                ecol[m, s, cols[p]] = 1.0
                erow[m, s, rows[p]] = 1.0

    woo_pad = np.zeros((H, OFFP), f)
    woo_pad[:, :OFF] = np.asarray(inputs["Woo"], f)
    boo_v = np.asarray(inputs["boo"], f)
    blobb = np.zeros((128, 256), f)
    blobb[rows, cols] = boo_v                     # b1: v_c += boo_rc * x_r
    blobb[cols, 128 + rows] = boo_v               # b2: out_r += boo_rc * v_c

    def bt2(v):  # [256] -> [128, 2]
        return np.asarray(v, f).reshape(2, 128).T

    blob = np.zeros((128, 9 + BT), f)
    blob[:, 0:2] = bt2(inputs["bd1"])
    blob[:, 2:4] = bt2(inputs["bo1"])
    blob[:, 4:6] = bt2(inputs["bd2"])
    blob[:, 6:8] = bt2(inputs["bo2"])
    blob[:N, 8] = np.asarray(inputs["bdo"], f)
    blob[:N, 9:] = np.asarray(inputs["damp_min"], f).reshape(N, 1)

    def pad1(w):  # [64, M] -> [128, M] zero-padded
        w = np.asarray(w, f)
        out = np.zeros((128, w.shape[1]), f)
        out[:N] = w
        return out

    def kt(w):  # [256, M] -> [128, 2, M]
        w = np.asarray(w, f)
        return np.ascontiguousarray(w.reshape(2, 128, -1).transpose(1, 0, 2))

    def bt(v):  # [256] -> [128, 2]
        return np.ascontiguousarray(np.asarray(v, f).reshape(2, 128).T)

    return {
        "wd1": pad1(inputs["Wd1"]).astype(bf),
        "wd2": kt(inputs["Wd2"]).astype(bf),
        "wdo": kt(np.concatenate(
            [np.asarray(inputs["Wdo"], f), np.zeros((H, 128 - N), f)],
            axis=1)).astype(bf),
        "wo1": pad1(inputs["Wo1"]).astype(bf),
        "wo2": kt(inputs["Wo2"]).astype(bf),
        "woo": kt(woo_pad).astype(bf),
        "blob": blob,
        "blobb": blobb.astype(bf),
        "rcol": rcol.astype(bf),
        "ecol": np.ascontiguousarray(
            ecol.reshape(SL, NSLICES * 128)).astype(bf),
        "erow": np.ascontiguousarray(
            erow.reshape(SL, NSLICES * 128)).astype(bf),
    }


def kernel(trace=False, **inputs):
    import ml_dtypes
    from concourse.bass_utils import run_bass_kernel_spmd

    nc = _get_program(with_boo=bool(np.any(np.asarray(inputs["boo"]))))
    consts = _host_consts(inputs)
    wd1 = consts.pop("wd1")
    wo1 = consts.pop("wo1")
    xt = np.asarray(inputs["x"], np.float32).T.astype(ml_dtypes.bfloat16)
    rows, _ = np.tril_indices(N, k=-1)
    rows_p = np.concatenate([rows, np.zeros(OFFP - len(rows), int)])
    in_maps = []
    for i in range(NCORES):
        xt_c = np.zeros((128, BLOCAL), ml_dtypes.bfloat16)
        xt_c[:N] = xt[:, i * BLOCAL:(i + 1) * BLOCAL]
        xe1_c = np.ascontiguousarray(
            xt_c[rows_p].reshape(NSLICES, SL, BLOCAL).transpose(1, 0, 2))
        w1x_c = np.ascontiguousarray(
            np.concatenate([wd1, wo1, xt_c[:, :BT]], axis=1))
        in_maps.append({"xt": xt_c, "w1x": w1x_c, "xe1": xe1_c, **consts})
    res = run_bass_kernel_spmd(nc, in_maps, core_ids=list(range(NCORES)),
                               trace=trace)
    out = np.concatenate(
        [np.ascontiguousarray(res.results[i]["out"].T) for i in range(NCORES)],
        axis=0)
    if trace:
        kernel.last_results = res
    return out


# revision 78
# speedup vs baseline: 1.0027x; 1.0027x over previous
"""Trainium2 Bass kernel for nn_Damping (B=32768, N=64, H=256).

Per-sample computation:
    diag = (relu(MLP_d(x)) + damp_min) * x          # [64]
    off  = MLP_o(x)                                  # [2016] strictly-lower entries
    L    = scatter(off -> strict lower, diag -> diagonal)   # [64, 64]
    out  = L @ (L^T @ x)

Strategy: pure data parallel over 8 NeuronCores (4096 samples each).
On-chip layout is feature-major: x arrives pre-transposed from the host as
bf16 [64, 4096] and the output leaves feature-major [64, 4096] f32 (host
transposes back), so the device does zero PE transposes. The scatter
matvecs avoid materializing L:
    v   = Ecol^T @ (off * (Rrow @ xT)) + diag * x       (v = L^T x)
    out = Erow^T @ (off * (Rcol @ vT)) + diag * v       (out = L v)
with Rrow/Rcol 0/1 expansion matrices and Ecol/Erow 0/1 reduction matrices
(PE matmuls, fp32 PSUM accumulation). All matmul operands are bf16.

Per 512-sample block: 110 matmul passes (free=512). Emission is software-
pipelined so the PE queue never head-of-line blocks on the DVE multiplies:
reduction matmuls for slice-pair q are emitted after the independent
woo/expand matmuls of pair q+1. Elementwise work is split DVE (scatter
multiplies, PSUM-reading adds) / Act (PSUM->SBUF off copies + tanh) /
GpSimd (SBUF-only diag-path ops).
"""

import numpy as np

B, N, H, OFF = 32768, 64, 256, 2016
NCORES = 8
BLOCAL = B // NCORES          # 4096 samples per core
NSLICES = 16
SL = 128                      # padded slice width; 16*128 = 2048
OFFP = NSLICES * SL           # 2048 (padded off dim)
NBLOCKS = 8                   # blocks of 512 samples per core
BT = 512                      # batch tile (moving free dim)
NPAIRS = NSLICES // 2         # slice pairs for the paired DVE multiplies

_compiled = {}


def _build_program(with_boo=True):
    import concourse.bass as bass  # noqa: F401
    import concourse.mybir as mybir
    import concourse.tile as tile
    from concourse import bacc

    f32 = mybir.dt.float32
    bf16 = mybir.dt.bfloat16
    AF = mybir.ActivationFunctionType

    nc = bacc.Bacc("TRN2", target_bir_lowering=False, debug=False,
                   num_devices=NCORES)

    def din(name, shape, dt=f32):
        return nc.dram_tensor(name, list(shape), dt, kind="ExternalInput").ap()

    xt_ap = din("xt", (128, BLOCAL), bf16)     # bottom 64 partitions zero
    xe1_ap = din("xe1", (SL, NSLICES, BLOCAL), bf16)
    wd1_ap = din("wd1", (128, H), bf16)        # bottom 64 rows zero
    wd2_ap = din("wd2", (128, 2, H), bf16)
    wdo_ap = din("wdo", (128, 2, 128), bf16)   # out cols 64-127 zero
    wo1_ap = din("wo1", (128, H), bf16)        # bottom 64 rows zero
    wo2_ap = din("wo2", (128, 2, H), bf16)
    woo_ap = din("woo", (128, 2, OFFP), bf16)
    # small consts packed: cols 0-1 bd1, 2-3 bo1, 4-5 bd2, 6-7 bo2, 8 bdo,
    # 9..521 dmf (bdo/dmf live on partitions 0-63)
    blob_ap = din("blob", (128, 9 + BT))
    # b1 = Ecol^T diag(boo) Rrow, b2 = Erow^T diag(boo) Rcol (both padded)
    blobb_ap = din("blobb", (128, 256), bf16)
    rcol_ap = din("rcol", (128, OFFP), bf16)   # bottom 64 rows zero
    ecol_ap = din("ecol", (SL, NSLICES * 128), bf16)  # out cols 64-127 zero
    erow_ap = din("erow", (SL, NSLICES * 128), bf16)
    out_ap = nc.dram_tensor("out", [N, BLOCAL], f32, kind="ExternalOutput").ap()

    with tile.TileContext(nc) as tc:
        with (
            tc.tile_pool(name="consts", bufs=1) as consts,
            tc.tile_pool(name="acts", bufs=2) as act_pool,
            tc.tile_pool(name="offp", bufs=2) as off_pool,
            tc.tile_pool(name="mp", bufs=4) as m_pool,
            tc.tile_pool(name="small", bufs=2) as small_pool,
            tc.tile_pool(name="outp", bufs=2) as out_pool,
            tc.tile_pool(name="xe1", bufs=2) as xe_pool,
            # PSUM: 8 banks of [128, 512] f32 total.
            tc.tile_pool(name="ps_a", bufs=2, space="PSUM") as ps_a,      # 2
            tc.tile_pool(name="ps_big", bufs=2, space="PSUM") as ps_big,  # 4
            tc.tile_pool(name="ps_acc", bufs=2, space="PSUM") as ps_acc,  # 2
        ):
            # ---- load constants ----
            _ld_engines = [nc.sync, nc.scalar]
            _ld_n = [0]

            def load(name, shape, ap):
                t = consts.tile(list(shape), ap.dtype, tag=name, name=name)
                _ld_engines[_ld_n[0] % 2].dma_start(t[:], ap)
                _ld_n[0] += 1
                return t

            # Loads ordered by first use so the PE can start ~immediately.
            xts = []

            def load_xt(b):
                t = consts.tile([128, BT], bf16, tag=f"xt{b}", name=f"xt{b}")
                nc.sync.dma_start(t[:], xt_ap[:, BT * b:BT * (b + 1)])
                xts.append(t)

            # double-buffered HBM-precomputed pass-1 expansion tiles
            xe_tiles = [None] * NBLOCKS

            def prefetch_xe(b):
                if b < NBLOCKS:
                    t = xe_pool.tile([SL, NSLICES, BT], bf16, tag="xe1")
                    nc.sync.dma_start(t[:], xe1_ap[:, :, BT * b:BT * (b + 1)])
                    xe_tiles[b] = t

            wd1 = load("wd1", (128, H), wd1_ap)
            load_xt(0)
            wo1 = load("wo1", (128, H), wo1_ap)
            blob = load("blob", (128, 9 + BT), blob_ap)
            wd2 = load("wd2", (128, 2, H), wd2_ap)
            wo2 = load("wo2", (128, 2, H), wo2_ap)
            wdo = load("wdo", (128, 2, 128), wdo_ap)
            woo = load("woo", (128, 2, OFFP), woo_ap)
            blobb = load("blobb", (128, 256), blobb_ap)
            ecol = load("ecol", (SL, NSLICES * 128), ecol_ap)
            prefetch_xe(0)
            load_xt(1)
            rcol = load("rcol", (128, OFFP), rcol_ap)
            erow = load("erow", (SL, NSLICES * 128), erow_ap)
            prefetch_xe(1)
            for _b in range(2, NBLOCKS):
                load_xt(_b)
            bd1, bo1 = blob[:, 0:2], blob[:, 2:4]
            bd2, bo2 = blob[:, 4:6], blob[:, 6:8]
            bdo = blob[0:N, 8:9]
            dmf = blob[0:N, 9:9 + BT]
            b1, b2 = blobb[:, 0:128], blobb[:, 128:256]

            # v tiles: [128, BT] with the bottom 64 partitions kept zero so
            # the zero-padded 128-row rcol stationaries see finite data.
            vts = [consts.tile([128, BT], bf16, tag=f"v{i}", name=f"v{i}")
                   for i in (0, 1)]
            for vt in vts:
                nc.vector.tensor_copy(vt[N:128, :], xts[0][N:128, :])

            def mlp2(w1, b1, w2, b2, xT, tag):
                """Two tanh layers; returns [128, 2, 512] feature-major bf16.

                Emits only the L1 matmuls + activations; L2 is a second call
                so the two MLPs' matmuls interleave (PE never waits on tanh).
                """
                a1 = act_pool.tile([128, 2, BT], bf16, tag=tag + "1")
                for s in range(2):
                    ps = ps_a.tile([128, BT], f32, tag="mlp")
                    nc.tensor.matmul(ps[:], w1[:, 128 * s:128 * (s + 1)],
                                     xT, start=True, stop=True)
                    nc.scalar.activation(a1[:, s], ps[:], AF.Tanh,
                                         bias=b1[:, s:s + 1])
                a2 = act_pool.tile([128, 2, BT], bf16, tag=tag + "2")
                for s in range(2):
                    ps = ps_a.tile([128, BT], f32, tag="mlp")
                    for k in range(2):
                        nc.tensor.matmul(ps[:], w2[:, k, 128 * s:128 * (s + 1)],
                                         a1[:, k], start=(k == 0), stop=(k == 1))
                    nc.scalar.activation(a2[:, s], ps[:], AF.Tanh,
                                         bias=b2[:, s:s + 1])
                return a2

            def scatter_pass1(off, xe, g2, acc_ps, mov, pending=None):
                """off = Woo@g2 (boo folded into acc via B1);
                acc = Ecol^T (off * xe) + B1 @ x. The pass-1 expansion xe is
                precomputed on the host and streamed from HBM, so the multiply
                is all-SBUF bf16 (2x DVE mode). Reduction matmuls for pair q
                are emitted inside iteration q+1 so the PE queue doesn't block
                on the DVE."""
                if with_boo:
                    nc.tensor.matmul(acc_ps[:], b1, mov,
                                     start=True, stop=False)
                m1s = [None] * NPAIRS
                for q in range(NPAIRS):
                    if q == 1 and pending is not None:
                        pending()
                    pso = ps_big.tile([128, 2 * BT], f32, tag="big")
                    for j in range(2):
                        s = 2 * q + j
                        for k in range(2):
                            nc.tensor.matmul(
                                pso[:, BT * j:BT * (j + 1)],
                                woo[:, k, SL * s:SL * (s + 1)],
                                g2[:, k], start=(k == 0), stop=(k == 1))
                    nc.scalar.copy(off[:, 2 * q:2 * q + 2], pso[:])
                    # reductions delayed by two pairs (PE never waits on DVE)
                    if q > 1:
                        for j in range(2):
                            s = 2 * (q - 2) + j
                            nc.tensor.matmul(
                                acc_ps[:], ecol[:, 128 * s:128 * (s + 1)],
                                m1s[q - 2][:, BT * j:BT * (j + 1)],
                                start=(not with_boo and s == 0), stop=False)
                    m1 = m_pool.tile([128, 2 * BT], bf16, tag="m1")
                    m1s[q] = m1
                    nc.vector.tensor_mul(out=m1[:], in0=off[:, 2 * q:2 * q + 2],
                                         in1=xe[:, 2 * q:2 * q + 2])

                def finish():
                    for q in (NPAIRS - 2, NPAIRS - 1):
                        for j in range(2):
                            s = 2 * q + j
                            nc.tensor.matmul(
                                acc_ps[:], ecol[:, 128 * s:128 * (s + 1)],
                                m1s[q][:, BT * j:BT * (j + 1)],
                                start=False, stop=(s == OFFP // SL - 1))
                return finish

            def scatter_pass2(off, mov, acc_ps):
                """acc = Erow^T (off * (Rcol @ mov)) + B2 @ mov."""
                if with_boo:
                    nc.tensor.matmul(acc_ps[:], b2, mov,
                                     start=True, stop=False)
                m1s = [None] * NPAIRS
                for q in range(NPAIRS):
                    pse = ps_big.tile([128, 2 * BT], f32, tag="big")
                    for j in range(2):
                        s = 2 * q + j
                        nc.tensor.matmul(
                            pse[:, BT * j:BT * (j + 1)],
                            rcol[:, SL * s:SL * (s + 1)],
                            mov, start=True, stop=True)
                    if q > 1:
                        for j in range(2):
                            s = 2 * (q - 2) + j
                            nc.tensor.matmul(
                                acc_ps[:], erow[:, 128 * s:128 * (s + 1)],
                                m1s[q - 2][:, BT * j:BT * (j + 1)],
                                start=(not with_boo and s == 0), stop=False)
                    m1 = m_pool.tile([128, 2 * BT], bf16, tag="m2")
                    m1s[q] = m1
                    nc.vector.tensor_mul(out=m1[:], in0=off[:, 2 * q:2 * q + 2],
                                         in1=pse[:])

                def finish():
                    for q in (NPAIRS - 2, NPAIRS - 1):
                        for j in range(2):
                            s = 2 * q + j
                            nc.tensor.matmul(
                                acc_ps[:], erow[:, 128 * s:128 * (s + 1)],
                                m1s[q][:, BT * j:BT * (j + 1)],
                                start=False, stop=(s == OFFP // SL - 1))
                return finish

            def mlp_block(b, pending=None):
                """Both MLPs for block b (matmuls interleaved). `pending`
                (deferred tail reductions of the previous pass) is emitted
                between the two MLPs so those matmuls never head-of-line
                block the PE queue while their DVE inputs finish."""
                xT = xts[b][:]
                h2 = mlp2(wd1, bd1, wd2, bd2, xT, "h")
                if pending is not None:
                    pending()
                g2 = mlp2(wo1, bo1, wo2, bo2, xT, "g")
                return h2, g2

            mlps = mlp_block(0)
            fin2 = None
            for b in range(NBLOCKS):
                xT = xts[b][:]                          # [128, BT], bottom 0
                xTn = xts[b][0:N, :]                    # [64, BT] top view
                h2, g2 = mlps

                # ---- diag = (relu(d + bdo) + dm) * x  (fp32) ----
                psd = ps_a.tile([128, BT], f32, tag="mlp")
                for k in range(2):
                    nc.tensor.matmul(psd[:], wdo[:, k, :], h2[:, k],
                                     start=(k == 0), stop=(k == 1))
                dr = small_pool.tile([N, BT], f32, tag="dr")
                nc.scalar.activation(dr[:], psd[0:N, :], AF.Relu, bias=bdo)
                dd = small_pool.tile([N, BT], f32, tag="dd")
                nc.gpsimd.tensor_add(out=dd[:], in0=dr[:], in1=dmf)
                diag = small_pool.tile([N, BT], f32, tag="diag")
                nc.gpsimd.tensor_mul(out=diag[:], in0=dd[:], in1=xTn)
                dvx = small_pool.tile([N, BT], f32, tag="dvx")
                nc.gpsimd.tensor_mul(out=dvx[:], in0=diag[:], in1=xTn)

                # ---- pass 1: v = Ecol^T (off * xe) + B1 x + diag*x ----
                off = off_pool.tile([SL, NSLICES, BT], bf16, tag="off")
                psv = ps_acc.tile([128, BT], f32, tag="acc")
                prefetch_xe(b + 2)
                fin1 = scatter_pass1(off, xe_tiles[b], g2, psv, xT,
                                     pending=fin2)

                # next block's MLP matmuls fill the PE while v is assembled;
                # pass-1 tail reductions are emitted inside (never at queue
                # head while their DVE multiplies finish)
                if b + 1 < NBLOCKS:
                    mlps = mlp_block(b + 1, pending=fin1)
                else:
                    fin1()
                v = vts[b % 2]
                nc.vector.tensor_add(out=v[0:N, :], in0=psv[0:N, :],
                                     in1=dvx[:])

                # ---- pass 2: out = Erow^T (off * (Rcol vT)) + B2 v + diag*v
                pso2 = ps_acc.tile([128, BT], f32, tag="acc")
                fin2t = scatter_pass2(off, v[:], pso2)
                dvv = small_pool.tile([N, BT], f32, tag="dvv")
                nc.gpsimd.tensor_mul(out=dvv[:], in0=diag[:], in1=v[0:N, :])

                def out_emit(b=b, pso2=pso2, dvv=dvv, fin2t=fin2t):
                    fin2t()   # close the pso2 accumulation group first
                    outf = out_pool.tile([N, BT], f32, tag="outf",
                                         name="outf")
                    nc.vector.tensor_add(out=outf[:], in0=pso2[0:N, :],
                                         in1=dvv[:])
                    nc.sync.dma_start(out_ap[:, BT * b:BT * (b + 1)],
                                      outf[:])

                if b == NBLOCKS - 1:
                    out_emit()
                else:
                    fin2 = out_emit

    nc.compile()
    return nc


def _get_program(with_boo=True):
    if with_boo not in _compiled:
        _compiled[with_boo] = _build_program(with_boo)
    return _compiled[with_boo]


def _host_consts(inputs):
    import ml_dtypes
    f = np.float32
    bf = ml_dtypes.bfloat16
    rows, cols = np.tril_indices(N, k=-1)         # length 2016
    # padded index arrays: entries p >= 2016 are dead (all matrices zero there)
    npad = OFFP - len(rows)                        # 32

    def onehot(idx, num, valid):
        m = np.zeros((num, OFFP), f)
        m[idx[valid], np.where(valid)[0]] = 1.0
        return m

    valid = np.ones(OFFP, bool)
    valid[len(rows):] = False
    cols_p = np.concatenate([cols, np.zeros(npad, int)])

    rcol = np.zeros((128, OFFP), f)
    rcol[:N] = onehot(cols_p, N, valid)           # padded [128, 2048]
    ecol = np.zeros((SL, NSLICES, 128), f)
    erow = np.zeros((SL, NSLICES, 128), f)
    for s in range(NSLICES):
        for m in range(SL):
            p = SL * s + m
            if p < len(rows):
                ecol[m, s, cols[p]] = 1.0
                erow[m, s, rows[p]] = 1.0

    woo_pad = np.zeros((H, OFFP), f)
    woo_pad[:, :OFF] = np.asarray(inputs["Woo"], f)
    boo_v = np.asarray(inputs["boo"], f)
    blobb = np.zeros((128, 256), f)
    blobb[rows, cols] = boo_v                     # b1: v_c += boo_rc * x_r
    blobb[cols, 128 + rows] = boo_v               # b2: out_r += boo_rc * v_c

    def bt2(v):  # [256] -> [128, 2]
        return np.asarray(v, f).reshape(2, 128).T

    blob = np.zeros((128, 9 + BT), f)
    blob[:, 0:2] = bt2(inputs["bd1"])
    blob[:, 2:4] = bt2(inputs["bo1"])
    blob[:, 4:6] = bt2(inputs["bd2"])
    blob[:, 6:8] = bt2(inputs["bo2"])
    blob[:N, 8] = np.asarray(inputs["bdo"], f)
    blob[:N, 9:] = np.asarray(inputs["damp_min"], f).reshape(N, 1)

    def pad1(w):  # [64, M] -> [128, M] zero-padded
        w = np.asarray(w, f)
        out = np.zeros((128, w.shape[1]), f)
        out[:N] = w
        return out

    def kt(w):  # [256, M] -> [128, 2, M]
        w = np.asarray(w, f)
        return np.ascontiguousarray(w.reshape(2, 128, -1).transpose(1, 0, 2))

    def bt(v):  # [256] -> [128, 2]
        return np.ascontiguousarray(np.asarray(v, f).reshape(2, 128).T)

    return {
        "wd1": pad1(inputs["Wd1"]).astype(bf),
        "wd2": kt(inputs["Wd2"]).astype(bf),
        "wdo": kt(np.concatenate(
            [np.asarray(inputs["Wdo"], f), np.zeros((H, 128 - N), f)],
            axis=1)).astype(bf),
        "wo1": pad1(inputs["Wo1"]).astype(bf),
        "wo2": kt(inputs["Wo2"]).astype(bf),
        "woo": kt(woo_pad).astype(bf),
        "blob": blob,
        "blobb": blobb.astype(bf),
        "rcol": rcol.astype(bf),
        "ecol": np.ascontiguousarray(
            ecol.reshape(SL, NSLICES * 128)).astype(bf),
        "erow": np.ascontiguousarray(
            erow.reshape(SL, NSLICES * 128)).astype(bf),
    }


def kernel(trace=False, **inputs):
    import ml_dtypes
    from concourse.bass_utils import run_bass_kernel_spmd

    nc = _get_program(with_boo=bool(np.any(np.asarray(inputs["boo"]))))
    consts = _host_consts(inputs)
    xt = np.asarray(inputs["x"], np.float32).T.astype(ml_dtypes.bfloat16)
    rows, _ = np.tril_indices(N, k=-1)
    rows_p = np.concatenate([rows, np.zeros(OFFP - len(rows), int)])
    in_maps = []
    for i in range(NCORES):
        xt_c = np.zeros((128, BLOCAL), ml_dtypes.bfloat16)
        xt_c[:N] = xt[:, i * BLOCAL:(i + 1) * BLOCAL]
        xe1_c = np.ascontiguousarray(
            xt_c[rows_p].reshape(NSLICES, SL, BLOCAL).transpose(1, 0, 2))
        in_maps.append({"xt": xt_c, "xe1": xe1_c, **consts})
    res = run_bass_kernel_spmd(nc, in_maps, core_ids=list(range(NCORES)),
                               trace=trace)
    out = np.concatenate(
        [np.ascontiguousarray(res.results[i]["out"].T) for i in range(NCORES)],
        axis=0)
    if trace:
        kernel.last_results = res
    return out
